# revision 20
# baseline (speedup 1.0000x reference)
"""Multi-head self-attention (B=16, N=1024, D=768, H=12) on 8 TRN2 NeuronCores.

Data-parallel over batch (2 batches per core, weights replicated, no
collectives). v3: fp8 DoubleRow AV + host-side layout prep + batched
normalization. (v2: 354us, PE 92% busy; v1: 453.8us.)

Layout: token 8p+t lives at partition p, slot t (attention is
permutation-invariant over tokens; undone at the out DMA). xT arrives
PRE-TRANSPOSED from host ([6,128,1024] f16 per batch) -- the on-chip
transpose matmuls + their PSUM evacuations are gone. bqkvT ([128,18]
f32) and the fused proj bias (W_proj^T b_v + b_proj) also come from
host.

Attention per head pair: S^T = K Q^T (f16, two K=64 matmuls run
concurrently in the top/bottom PE halves), E = exp(S^T*scale) written
DIRECTLY AS FP8-E4M3 by ACT (values in (0, 12.6] -- far from the 240
clip; measured end-to-end rel err 1.6e-2 vs the 2e-2 gate, dominated
equally by the E and V casts -- Q/K/proj must stay f16: simulated
V-GEMM-in-f8 fails at 2.4e-2).  AV runs in DoubleRow perf mode: V_aug
(= [x W_v | ones-col], fp8, 65 cols per head) for m-steps 2j/2j+1 sits
in one [128, 2, 65] stationary tile (two k-planes per PE cell), E pair
in one [128, 2, 512] moving AP -> K=256 per matmul, HALF the AV slots
of f16. DoubleRow tile_size stays (128,128) = the filler GEMM mode, so
no extra array-reconfig tax (the measured dead end: 4 tile modes
cycling cost +50us at f16).

Normalization (the v2 chain burned ~3.7us DVE/pair in single-lane
[1,N] ops): the ones-row denominator is evacuated as row 64 of the
[65,512] f32 u2 PSUM-evac copy (free), DMA'd from SBUF through DRAM
into a [128,512] broadcast, then ONE 128-lane reciprocal + two muls.
recip/muls are deferred one pair so the DVE FIFO never parks on the
DMA round-trip. All f32 (reciprocal_approx_fast requires fp32 bit
layout).

Engine budget at 354us baseline: PE 331.8us busy (91.5%), ACT 218.4
(192 exps at (1024+352)/1.2 = 1147ns each -- the hard floor of this
decomposition: exp-batching to N=2048 needs 8 PSUM banks for sps
double-buffering, impossible next to ot+fillers), DVE 195.9. v3
targets: PE ~245 (DR-AV -36, transposes -5.5, bfinal -2.5), DVE ~150,
ACT ~212; both attention windows become ACT(exp)-bound at ~9.2us/pair.

Filler schedule: attention-0 hosts b0 Q/K groups + b1 V + early b1
Q/K triples (qk tags are per-batch now -- no cross-batch WAR);
attention-1 hosts remaining b1 Q/K + all b0 proj + b1 proj t0-3
(needs at[1] nh0 + its deferred mul -> placed after pop 28); tail is
b1 proj t4-7 with out-DMAs spread across sync/vector/scalar queues.
DMA issue costs ~0.6us of the ISSUING engine: weights/per-pair
denominator round-trips ride gpsimd; xT0 splits across sync/scalar/
vector for a parallel ~6us prologue; nothing issues from scalar after
the first exp (ACT is the floor).
"""

import numpy as np

_CACHE: dict = {}

P = 128
BL, N, D, H, HD = 2, 1024, 768, 12, 64
D3 = 3 * D
SCALE = float(HD) ** -0.5


def _build():
    import concourse.mybir as mybir
    import concourse.tile as tile
    from concourse import bacc

    dt = mybir.dt
    F32, F16, F8 = dt.float32, dt.float16, dt.float8e4
    AF = mybir.ActivationFunctionType
    DR = mybir.MatmulPerfMode.DoubleRow

    nc = bacc.Bacc("TRN2", target_bir_lowering=False, debug=False)
    xT_d = nc.dram_tensor("xT", [BL, 6, P, N], F16, kind="ExternalInput").ap()
    wqkv_d = nc.dram_tensor("W_qkv", [D, D3], F16, kind="ExternalInput").ap()
    bqkvT_d = nc.dram_tensor("bqkvT", [P, 18], F32, kind="ExternalInput").ap()
    wproj_d = nc.dram_tensor("W_proj", [D, D], F16, kind="ExternalInput").ap()
    bfin_d = nc.dram_tensor("bfin", [1, D], F32, kind="ExternalInput").ap()
    out_d = nc.dram_tensor("out", [BL, N, D], F16, kind="ExternalOutput").ap()
    # token-interleaved view: partition p, slot t <-> token 8p+t
    out_il = out_d.rearrange("b (p i) d -> b i p d", p=P)     # [2, 8, 128, 768]

    with tile.TileContext(nc) as tc:
        with tc.tile_pool(name="sb", bufs=1) as sb, \
             tc.tile_pool(name="dp", bufs=1, space="DRAM") as dp, \
             tc.tile_pool(name="ps", bufs=2, space="PSUM") as ps:

            # ---------- constants ----------
            ones_h = sb.tile([P, P], F16, tag="ones_h", bufs=1, name="ones_h")
            nc.vector.memset(ones_h[:], 1.0)
            warm_h = sb.tile([P, 512], F16, tag="warm", bufs=1, name="warm_h")
            nc.vector.memset(warm_h[:], 0.0)

            # ---------- input DMAs ----------
            bqkvT = sb.tile([P, 18], F32, tag="bqkvT", bufs=1, name="bqkvT")
            nc.sync.dma_start(bqkvT[:], bqkvT_d)
            xT = {b: [sb.tile([P, N], F16, tag=f"xT{b}_{j}", bufs=1,
                              name=f"xT{b}_{j}") for j in range(6)]
                  for b in range(BL)}
            wq_h, wv_h, wp_h = [], [], []
            # W_v first on gpsimd (V groups are the prologue compute)
            for d in range(6):
                t = sb.tile([P, D], F16, tag=f"wv{d}", bufs=1, name=f"wv{d}")
                nc.gpsimd.dma_start(t[:], wqkv_d[P * d:P * (d + 1), 2 * D:D3])
                wv_h.append(t)
            # xT0 split across sync/scalar queues (both idle in prologue;
            # DMA issue is restricted to gpsimd/sync/scalar). HWDGE issue
            # instructions BLOCK the issuing engine until the ring frees
            # (~one 384KB transfer = ~4us), so scalar carries only
            # prologue-critical transfers and is clear before the first exp.
            for d in range(6):
                (nc.sync if d < 3 else nc.scalar).dma_start(
                    xT[0][d][:], xT_d[0, d])
            # W_q/W_k spread over all three queues (pair-0 gate)
            wq_q = (nc.sync, nc.sync, nc.scalar, nc.scalar,
                    nc.gpsimd, nc.gpsimd)
            for d in range(6):
                t = sb.tile([P, 2 * D], F16, tag=f"wqkv{d}", bufs=1,
                            name=f"wqkv{d}")
                wq_q[d].dma_start(t[:], wqkv_d[P * d:P * (d + 1), 0:2 * D])
                wq_h.append(t)
            # xT1 on gpsimd (needed only mid-attention-0)
            for d in range(6):
                nc.gpsimd.dma_start(xT[1][d][:], xT_d[1, d])
            for d in range(6):
                t = sb.tile([P, D], F16, tag=f"wproj{d}", bufs=1,
                            name=f"wproj{d}")
                nc.gpsimd.dma_start(t[:], wproj_d[P * d:P * (d + 1), :])
                wp_h.append(t)
            bfin_bc = sb.tile([P, D], F32, tag="bfin_bc", bufs=1,
                              name="bfin_bc")
            nc.gpsimd.dma_start(bfin_bc[:], bfin_d.to_broadcast((P, D)))

            def keep_warm(n):
                # ~215ns each of dense matmul keeps/flips HAM to 8/8
                for wi in range(n):
                    wps = ps.tile([P, 512], F32, tag="mm", bufs=2, name="wps")
                    nc.tensor.matmul(wps[:], ones_h[:, 0:P], warm_h[:],
                                     start=True, stop=True)

            # PE warm-up bridges the DGE-ramp / DMA wait and flips HAM
            keep_warm(18)
            # pre-load the exp table set (~2.7us ACT) during the DMA wait
            escr = sb.tile([P, 16], F8, tag="escr", bufs=1, name="escr")
            nc.scalar.activation(escr[:], warm_h[:, 0:16], AF.Exp, scale=1.0)
            keep_warm(8)

            # ---------- tiles ----------
            # qk tags per-batch: b1 Q/K groups may run inside attention-0
            qk = {b: [sb.tile([P, N], F16, tag=f"qk{b}_{j}", bufs=1,
                              name=f"qk{b}_{j}") for j in range(12)]
                  for b in range(BL)}
            # V pairs, fp8, plane-major: [2 m-planes x 12 heads x 80] --
            # 80-wide head blocks (65 used) keep the DoubleRow weights AP
            # 16-byte aligned (plane stride 960, head offset 80h; walrus
            # ISA-checks the LDWEIGHTS access pattern)
            vp = {b: [sb.tile([P, 2 * H * 80], F8, tag=f"v{b}_{j}", bufs=1,
                              name=f"v{b}_{j}") for j in range(4)]
                  for b in range(BL)}
            at = {b: [sb.tile([P, N], F16, tag=f"at{b}_{j}", bufs=1,
                              name=f"at{j}") for j in range(6)]
                  for b in range(BL)}

            def emit_qkv_group(b, j, nh, tag="mm"):
                qps = ps.tile([P, 512], F32, tag=tag, bufs=2, name="qps")
                for d in range(6):
                    nc.tensor.matmul(qps[:], wq_h[d][:, P * j:P * (j + 1)],
                                     xT[b][d][:, 512 * nh:512 * (nh + 1)],
                                     start=(d == 0), stop=(d == 5))
                nc.vector.tensor_scalar_add(
                    qk[b][j][:, 512 * nh:512 * (nh + 1)], qps[:],
                    bqkvT[:, j:j + 1])

            def emit_v_group(b, t, ci):
                c0, cw = ((0, 512), (512, 256))[ci]
                v4 = vp[b][t // 2].rearrange("p (two h m) -> p two h m",
                                             two=2, m=80)
                if ci == 0:
                    # ones columns for this plane (the denominator rows)
                    nc.vector.tensor_copy(v4[:, t % 2, :, 64:65],
                                          ones_h[:, 0:H].unsqueeze(2))
                vps = ps.tile([P, 512], F32, tag="mm", bufs=2, name="vps")
                for d in range(6):
                    nc.tensor.matmul(vps[:, 0:cw], xT[b][d][:, P * t:P * (t + 1)],
                                     wv_h[d][:, c0:c0 + cw],
                                     start=(d == 0), stop=(d == 5))
                nc.vector.tensor_copy(
                    v4[:, t % 2, 8 * ci:8 * ci + cw // HD, 0:HD],
                    vps[:, 0:cw].rearrange("p (h c) -> p h c", c=HD))

            def emit_proj_half(b, t, ci, queue=None):
                c0, cw = ((0, 512), (512, 256))[ci]
                pps = ps.tile([P, 512], F32, tag="mm", bufs=2, name="pps")
                for d in range(6):
                    nc.tensor.matmul(pps[:, 0:cw],
                                     at[b][d][:, P * t:P * (t + 1)],
                                     wp_h[d][:, c0:c0 + cw],
                                     start=(d == 0), stop=(d == 5))
                osb = sb.tile([P, 512], F16, tag="outs", bufs=2, name="osb")
                nc.vector.tensor_add(osb[:, 0:cw], pps[:, 0:cw],
                                     bfin_bc[:, c0:c0 + cw])
                (queue or nc.sync).dma_start(out_il[b, t][:, c0:c0 + cw],
                                             osb[:, 0:cw])

            # deferred normalization state: (emit recip+muls of pair p at
            # pair p+1's end, once the broadcast DMA round-trip has landed)
            pending_norm = [None]

            def flush_norm():
                if pending_norm[0] is not None:
                    pending_norm[0]()
                    pending_norm[0] = None

            def emit_attention(b, fillers, pops_sched):
                for nh in range(2):
                    n0 = 512 * nh
                    for jp in range(6):
                        pops = pops_sched[6 * nh + jp]
                        qt, kt = qk[b][jp], qk[b][6 + jp]
                        ot = [ps.tile([65, 512], F32, tag="ot", bufs=2,
                                      name="otps") for _ in range(2)]
                        es2 = []

                        def do_av(j):
                            # one DoubleRow matmul per head: K=256 (two
                            # 128-token planes per PE cell), fp8 operands
                            e2 = es2[j].rearrange("p (two x) -> p two x",
                                                  two=2)
                            v4 = vp[b][j].rearrange("p (two x) -> p two x",
                                                    two=2)
                            for hh in range(2):
                                h = 2 * jp + hh
                                nc.tensor.matmul(
                                    ot[hh][:],
                                    v4[:, :, 80 * h:80 * h + 65],
                                    e2[:, :, 512 * hh:512 * (hh + 1)],
                                    start=(j == 0), stop=(j == 3),
                                    perf_mode=DR)

                        for mb in range(4):
                            if mb > 0:
                                do_av(mb - 1)
                            sps2 = []
                            for mi in range(2):
                                m = 2 * mb + mi
                                sps = ps.tile([P, N], F32, tag="s", bufs=2,
                                              name="sps")
                                for hh in range(2):
                                    r0, r1 = HD * hh, HD * (hh + 1)
                                    nc.tensor.matmul(
                                        sps[:, 512 * hh:512 * (hh + 1)],
                                        kt[r0:r1, P * m:P * (m + 1)],
                                        qt[r0:r1, n0:n0 + 512],
                                        start=True, stop=True)
                                sps2.append(sps)
                            e2t = sb.tile([P, 2048], F8, tag="e", bufs=2,
                                          name="e2")
                            es2.append(e2t)
                            for mi in range(2):
                                nc.scalar.activation(
                                    e2t[:, 1024 * mi:1024 * (mi + 1)],
                                    sps2[mi][:], AF.Exp, scale=SCALE)
                            for _ in range(pops[mb]):
                                if fillers:
                                    fillers.pop(0)()
                        do_av(3)

                        last = b == 1 and nh == 1 and jp == 5
                        # previous pair's recip+muls first (its broadcast
                        # landed during this pair)
                        flush_norm()
                        if last:
                            # tail critical path: all on-chip (the DMA
                            # round-trip costs ~8us here and re-throttles
                            # HAM). O evac on ACT (idle now), denominator
                            # rows + recips on DVE, broadcast via two
                            # rank-1 PE matmuls into the dead sps banks.
                            # Warm MMs first: they run while the DVE chain
                            # computes, holding HAM at 8/8 for the tail.
                            keep_warm(10)
                            u2f = sb.tile([P, 512], F16, tag="u2f", bufs=1,
                                          name="u2f")
                            drf = sb.tile([1, 1024], F32, tag="drf", bufs=1,
                                          name="drf")
                            for hh in range(2):
                                nc.vector.tensor_copy(
                                    drf[0:1, 512 * hh:512 * (hh + 1)],
                                    ot[hh][64:65, :])
                                nc.scalar.copy(u2f[HD * hh:HD * (hh + 1), :],
                                               ot[hh][0:HD, :])
                            rrf = sb.tile([1, 1024], F32, tag="rrf", bufs=1,
                                          name="rrf")
                            nc.vector.reciprocal_approx_fast(out=rrf[:],
                                                             in_=drf[:])
                            rrh = sb.tile([1, 1024], F16, tag="rrh", bufs=1,
                                          name="rrh")
                            nc.vector.tensor_copy(rrh[:], rrf[:])
                            bcp = ps.tile([P, 512], F32, tag="s", bufs=2,
                                          name="bcp")
                            for hh in range(2):
                                nc.tensor.matmul(
                                    bcp[HD * hh:HD * (hh + 1), :],
                                    ones_h[0:1, 0:HD],
                                    rrh[0:1, 512 * hh:512 * (hh + 1)],
                                    start=True, stop=True)
                            nc.vector.tensor_mul(at[b][jp][:, n0:n0 + 512],
                                                 u2f[:], bcp[:])
                            continue
                        # evacuate O^T + denominator row per head (f32),
                        # then DMA the denominator rows out for broadcast
                        u2a = sb.tile([65, 512], F32, tag="u2a", bufs=2,
                                      name="u2a")
                        u2b = sb.tile([65, 512], F32, tag="u2b", bufs=2,
                                      name="u2b")
                        nc.vector.tensor_copy(u2a[:], ot[0][:])
                        nc.vector.tensor_copy(u2b[:], ot[1][:])
                        dr_d = dp.tile([1, 1024], F32, tag="dr_d", bufs=2,
                                       name="dr_d")
                        bca = sb.tile([HD, 512], F32, tag="bca", bufs=2,
                                      name="bca")
                        bcb = sb.tile([HD, 512], F32, tag="bcb", bufs=2,
                                      name="bcb")
                        nc.gpsimd.dma_start(dr_d[0:1, 0:512], u2a[64:65, :])
                        nc.gpsimd.dma_start(dr_d[0:1, 512:1024],
                                            u2b[64:65, :])
                        nc.gpsimd.dma_start(
                            bca[:], dr_d[0:1, 0:512].to_broadcast((HD, 512)))
                        nc.gpsimd.dma_start(
                            bcb[:],
                            dr_d[0:1, 512:1024].to_broadcast((HD, 512)))

                        def norm(jp=jp, n0=n0, u2a=u2a, u2b=u2b,
                                 bca=bca, bcb=bcb):
                            rca = sb.tile([HD, 512], F32, tag="rca", bufs=2,
                                          name="rca")
                            rcb = sb.tile([HD, 512], F32, tag="rcb", bufs=2,
                                          name="rcb")
                            nc.vector.reciprocal_approx_fast(out=rca[:],
                                                             in_=bca[:])
                            nc.vector.reciprocal_approx_fast(out=rcb[:],
                                                             in_=bcb[:])
                            nc.vector.tensor_mul(
                                at[b][jp][0:HD, n0:n0 + 512],
                                u2a[0:HD, :], rca[:])
                            nc.vector.tensor_mul(
                                at[b][jp][HD:P, n0:n0 + 512],
                                u2b[0:HD, :], rcb[:])
                        pending_norm[0] = norm

            # ---------- prologue compute for attention-0 ----------
            # Only what gates pair 0: V t0/t1 (-> vp[0][0], needed by
            # AV(0) at mb1) + the pair-0 Q/K triple. Woven keep-warm MMs
            # hold HAM through the DMA-paced stretch. The remaining b0 V
            # groups front-load into fill0 (popped hard in pair 0 so
            # vp[0][j] lands just ahead of AV(j)).
            for t in range(2):
                for ci in range(2):
                    emit_v_group(0, t, ci)
                keep_warm(1)
            emit_qkv_group(0, 0, 0)
            emit_qkv_group(0, 6, 0)
            emit_qkv_group(0, 6, 1)

            # ---------- attention-0 fillers (56 pops) ----------
            fill0 = []
            fill0 += [lambda t=t, ci=ci: emit_v_group(0, t, ci)
                      for t in range(2, 8) for ci in range(2)]
            for jp in range(1, 6):
                fill0 += [lambda j=jp: emit_qkv_group(0, j, 0),
                          lambda j=jp + 6: emit_qkv_group(0, j, 0),
                          lambda j=jp + 6: emit_qkv_group(0, j, 1)]
            fill0 += [lambda j=jp: emit_qkv_group(0, j, 1) for jp in range(6)]
            fill0 += [lambda t=t, ci=ci: emit_v_group(1, t, ci)
                      for t in range(8) for ci in range(2)]
            for jp in range(2):
                fill0 += [lambda j=jp: emit_qkv_group(1, j, 0),
                          lambda j=jp + 6: emit_qkv_group(1, j, 0),
                          lambda j=jp + 6: emit_qkv_group(1, j, 1)]
            fill0 += [lambda: emit_qkv_group(1, 2, 0)]
            fill0 += [lambda: keep_warm(1)] * 7

            # pair 0 front-loads the 12 remaining b0-V units (vp[0][j]
            # lands just before AV(j) reads it) + the jp1 Q/K triple
            pops0 = [[2, 2, 4, 7]] + [[1, 1, 1, 1]] * 11
            emit_attention(0, fill0, pops0)
            for f in fill0:
                f()

            # ---------- attention-1 fillers (48 pops) ----------
            fill1 = [lambda: emit_qkv_group(1, 8, 0),
                     lambda: emit_qkv_group(1, 8, 1)]
            for jp in range(3, 6):
                fill1 += [lambda j=jp: emit_qkv_group(1, j, 0),
                          lambda j=jp + 6: emit_qkv_group(1, j, 0),
                          lambda j=jp + 6: emit_qkv_group(1, j, 1)]
            fill1 += [lambda j=jp: emit_qkv_group(1, j, 1) for jp in range(6)]
            fill1 += [lambda t=t, ci=ci: emit_proj_half(0, t, ci)
                      for t in range(8) for ci in range(2)]
            # b1 proj t0-3 needs at[1] nh0 incl. the deferred pair-5 mul
            # (flushed at pair 6) -> placed after pop 28
            fill1 += [lambda t=t, ci=ci: emit_proj_half(1, t, ci)
                      for t in range(4) for ci in range(2)]
            fill1 += [lambda: keep_warm(1)] * 7

            pops1 = [[1, 1, 1, 1]] * 12
            emit_attention(1, fill1, pops1)
            for f in fill1:
                f()

            # ---------- tail: b1 proj second half ----------
            # NOT gpsimd: its SWDGE path drains at ~1/3 the HWDGE rate
            tailq = [nc.sync, nc.scalar, nc.sync, nc.scalar,
                     nc.sync, nc.scalar, nc.sync, nc.scalar]
            qi = 0
            for t in range(4, 8):
                for ci in range(2):
                    emit_proj_half(1, t, ci, queue=tailq[qi])
                    qi += 1
    nc.compile()
    return nc


def _get_nc():
    if "nc" not in _CACHE:
        _CACHE["nc"] = _build()
    return _CACHE["nc"]


def _in_maps(x, W_qkv, b_qkv, W_proj, b_proj):
    x16 = np.ascontiguousarray(x, dtype=np.float16)
    # xT[b, j, p, 128*t + pp] = x[b, 8*pp + t, 128*j + p]
    xT = np.ascontiguousarray(
        x16.reshape(-1, P, 8, D).transpose(0, 3, 2, 1)
           .reshape(-1, 6, P, 8 * P))
    bq32 = np.asarray(b_qkv, dtype=np.float32)
    bqkvT = np.ascontiguousarray(bq32.reshape(18, P).T)
    bfin = (np.asarray(W_proj, dtype=np.float32).T @ bq32[2 * D:]
            + np.asarray(b_proj, dtype=np.float32)).reshape(1, D)
    bfin = np.ascontiguousarray(bfin, dtype=np.float32)
    wq16 = np.ascontiguousarray(W_qkv, dtype=np.float16)
    wp16 = np.ascontiguousarray(W_proj, dtype=np.float16)
    return [
        {
            "xT": xT[2 * i:2 * i + 2],
            "W_qkv": wq16,
            "bqkvT": bqkvT,
            "W_proj": wp16,
            "bfin": bfin,
        }
        for i in range(8)
    ]


def kernel(x, W_qkv, b_qkv, W_proj, b_proj):
    from concourse.bass_utils import run_bass_kernel_spmd

    nc = _get_nc()
    in_maps = _in_maps(x, W_qkv, b_qkv, W_proj, b_proj)
    res = run_bass_kernel_spmd(nc, in_maps, core_ids=list(range(8)))
    return np.concatenate(
        [r["out"].astype(np.float32) for r in res.results], axis=0)


# revision 22
# speedup vs baseline: 1.1701x; 1.1701x over previous
"""Multi-head self-attention (B=16, N=1024, D=768, H=12) on 8 TRN2 NeuronCores.

Data-parallel over batch (2 batches per core, weights replicated, no
collectives). v3: fp8 DoubleRow AV + host-side layout prep + batched
normalization. (v2: 354us, PE 92% busy; v1: 453.8us.)

Layout: token 8p+t lives at partition p, slot t (attention is
permutation-invariant over tokens; undone at the out DMA). xT arrives
PRE-TRANSPOSED from host ([6,128,1024] f16 per batch) -- the on-chip
transpose matmuls + their PSUM evacuations are gone. bqkvT ([128,18]
f32) and the fused proj bias (W_proj^T b_v + b_proj) also come from
host.

Attention per head pair: S^T = K Q^T (f16, two K=64 matmuls run
concurrently in the top/bottom PE halves), E = exp(S^T*scale) written
DIRECTLY AS FP8-E4M3 by ACT (values in (0, 12.6] -- far from the 240
clip; measured end-to-end rel err 1.6e-2 vs the 2e-2 gate, dominated
equally by the E and V casts -- Q/K/proj must stay f16: simulated
V-GEMM-in-f8 fails at 2.4e-2).  AV runs in DoubleRow perf mode: V_aug
(= [x W_v | ones-col], fp8, 65 cols per head) for m-steps 2j/2j+1 sits
in one [128, 2, 65] stationary tile (two k-planes per PE cell), E pair
in one [128, 2, 512] moving AP -> K=256 per matmul, HALF the AV slots
of f16. DoubleRow tile_size stays (128,128) = the filler GEMM mode, so
no extra array-reconfig tax (the measured dead end: 4 tile modes
cycling cost +50us at f16).

Normalization (the v2 chain burned ~3.7us DVE/pair in single-lane
[1,N] ops): the ones-row denominator is evacuated as row 64 of the
[65,512] f32 u2 PSUM-evac copy (free), DMA'd from SBUF through DRAM
into a [128,512] broadcast, then ONE 128-lane reciprocal + two muls.
recip/muls are deferred one pair so the DVE FIFO never parks on the
DMA round-trip. All f32 (reciprocal_approx_fast requires fp32 bit
layout).

Engine budget at 354us baseline: PE 331.8us busy (91.5%), ACT 218.4
(192 exps at (1024+352)/1.2 = 1147ns each -- the hard floor of this
decomposition: exp-batching to N=2048 needs 8 PSUM banks for sps
double-buffering, impossible next to ot+fillers), DVE 195.9. v3
targets: PE ~245 (DR-AV -36, transposes -5.5, bfinal -2.5), DVE ~150,
ACT ~212; both attention windows become ACT(exp)-bound at ~9.2us/pair.

Filler schedule: attention-0 hosts b0 Q/K groups + b1 V + early b1
Q/K triples (qk tags are per-batch now -- no cross-batch WAR);
attention-1 hosts remaining b1 Q/K + all b0 proj + b1 proj t0-3
(needs at[1] nh0 + its deferred mul -> placed after pop 28); tail is
b1 proj t4-7 with out-DMAs spread across sync/vector/scalar queues.
DMA issue costs ~0.6us of the ISSUING engine: weights/per-pair
denominator round-trips ride gpsimd; xT0 splits across sync/scalar/
vector for a parallel ~6us prologue; nothing issues from scalar after
the first exp (ACT is the floor).
"""

import numpy as np

_CACHE: dict = {}

P = 128
BL, N, D, H, HD = 2, 1024, 768, 12, 64
D3 = 3 * D
SCALE = float(HD) ** -0.5


def _build():
    import concourse.mybir as mybir
    import concourse.tile as tile
    from concourse import bacc

    dt = mybir.dt
    F32, F16, F8 = dt.float32, dt.float16, dt.float8e4
    AF = mybir.ActivationFunctionType
    DR = mybir.MatmulPerfMode.DoubleRow

    nc = bacc.Bacc("TRN2", target_bir_lowering=False, debug=False)
    xT_d = nc.dram_tensor("xT", [BL, 6, P, N], F16, kind="ExternalInput").ap()
    wqkv_d = nc.dram_tensor("W_qkv", [D, D3], F16, kind="ExternalInput").ap()
    bqkvT_d = nc.dram_tensor("bqkvT", [P, 18], F32, kind="ExternalInput").ap()
    wproj_d = nc.dram_tensor("W_proj", [D, D], F16, kind="ExternalInput").ap()
    bfin_d = nc.dram_tensor("bfin", [1, D], F32, kind="ExternalInput").ap()
    out_d = nc.dram_tensor("out", [BL, N, D], F16, kind="ExternalOutput").ap()
    # token-interleaved view: partition p, slot t <-> token 8p+t
    out_il = out_d.rearrange("b (p i) d -> b i p d", p=P)     # [2, 8, 128, 768]

    with tile.TileContext(nc) as tc:
        with tc.tile_pool(name="sb", bufs=1) as sb, \
             tc.tile_pool(name="dp", bufs=1, space="DRAM") as dp, \
             tc.tile_pool(name="ps", bufs=2, space="PSUM") as ps:

            # ---------- constants ----------
            ones_h = sb.tile([P, P], F16, tag="ones_h", bufs=1, name="ones_h")
            nc.vector.memset(ones_h[:], 1.0)
            warm_h = sb.tile([P, 512], F16, tag="warm", bufs=1, name="warm_h")
            nc.vector.memset(warm_h[:], 0.0)

            # ---------- input DMAs ----------
            bqkvT = sb.tile([P, 18], F32, tag="bqkvT", bufs=1, name="bqkvT")
            nc.sync.dma_start(bqkvT[:], bqkvT_d)
            xT = {b: [sb.tile([P, N], F16, tag=f"xT{b}_{j}", bufs=1,
                              name=f"xT{b}_{j}") for j in range(6)]
                  for b in range(BL)}
            wq_h, wv_h, wp_h = [], [], []
            # W_v first on gpsimd (V groups are the prologue compute)
            for d in range(6):
                t = sb.tile([P, D], F16, tag=f"wv{d}", bufs=1, name=f"wv{d}")
                nc.gpsimd.dma_start(t[:], wqkv_d[P * d:P * (d + 1), 2 * D:D3])
                wv_h.append(t)
            # xT0 split across sync/scalar queues (both idle in prologue;
            # DMA issue is restricted to gpsimd/sync/scalar). HWDGE issue
            # instructions BLOCK the issuing engine until the ring frees
            # (~one 384KB transfer = ~4us), so scalar carries only
            # prologue-critical transfers and is clear before the first exp.
            for d in range(6):
                (nc.sync if d < 3 else nc.scalar).dma_start(
                    xT[0][d][:], xT_d[0, d])
            # W_q/W_k: 3 on sync, 3 on scalar (all issued pre-exp)
            for d in range(6):
                t = sb.tile([P, 2 * D], F16, tag=f"wqkv{d}", bufs=1,
                            name=f"wqkv{d}")
                (nc.sync if d < 3 else nc.scalar).dma_start(
                    t[:], wqkv_d[P * d:P * (d + 1), 0:2 * D])
                wq_h.append(t)
            # xT1: 3 on sync, 3 on gpsimd (needed only mid-attention-0)
            for d in range(6):
                (nc.sync if d < 3 else nc.gpsimd).dma_start(
                    xT[1][d][:], xT_d[1, d])
            for d in range(6):
                t = sb.tile([P, D], F16, tag=f"wproj{d}", bufs=1,
                            name=f"wproj{d}")
                nc.gpsimd.dma_start(t[:], wproj_d[P * d:P * (d + 1), :])
                wp_h.append(t)
            bfin_bc = sb.tile([P, D], F32, tag="bfin_bc", bufs=1,
                              name="bfin_bc")
            nc.gpsimd.dma_start(bfin_bc[:], bfin_d.to_broadcast((P, D)))

            def keep_warm(n):
                # ~215ns each of dense matmul keeps/flips HAM to 8/8
                for wi in range(n):
                    wps = ps.tile([P, 512], F32, tag="mm", bufs=2, name="wps")
                    nc.tensor.matmul(wps[:], ones_h[:, 0:P], warm_h[:],
                                     start=True, stop=True)

            # PE warm-up bridges the DGE-ramp / DMA wait and flips HAM
            keep_warm(18)
            # pre-load the exp table set (~2.7us ACT) during the DMA wait
            escr = sb.tile([P, 16], F8, tag="escr", bufs=1, name="escr")
            nc.scalar.activation(escr[:], warm_h[:, 0:16], AF.Exp, scale=1.0)
            keep_warm(8)

            # ---------- tiles ----------
            # qk tags per-batch: b1 Q/K groups may run inside attention-0
            qk = {b: [sb.tile([P, N], F16, tag=f"qk{b}_{j}", bufs=1,
                              name=f"qk{b}_{j}") for j in range(12)]
                  for b in range(BL)}
            # V pairs, fp8, plane-major: [2 m-planes x 12 heads x 80] --
            # 80-wide head blocks (65 used) keep the DoubleRow weights AP
            # 16-byte aligned (plane stride 960, head offset 80h; walrus
            # ISA-checks the LDWEIGHTS access pattern)
            vp = {b: [sb.tile([P, 2 * H * 80], F8, tag=f"v{b}_{j}", bufs=1,
                              name=f"v{b}_{j}") for j in range(4)]
                  for b in range(BL)}
            at = {b: [sb.tile([P, N], F16, tag=f"at{b}_{j}", bufs=1,
                              name=f"at{j}") for j in range(6)]
                  for b in range(BL)}

            def emit_qkv_group(b, j, nh, tag="mm"):
                qps = ps.tile([P, 512], F32, tag=tag, bufs=2, name="qps")
                for d in range(6):
                    nc.tensor.matmul(qps[:], wq_h[d][:, P * j:P * (j + 1)],
                                     xT[b][d][:, 512 * nh:512 * (nh + 1)],
                                     start=(d == 0), stop=(d == 5))
                nc.vector.tensor_scalar_add(
                    qk[b][j][:, 512 * nh:512 * (nh + 1)], qps[:],
                    bqkvT[:, j:j + 1])

            def emit_v_group(b, t, ci):
                c0, cw = ((0, 512), (512, 256))[ci]
                v4 = vp[b][t // 2].rearrange("p (two h m) -> p two h m",
                                             two=2, m=80)
                if ci == 0:
                    # ones columns for this plane (the denominator rows)
                    nc.vector.tensor_copy(v4[:, t % 2, :, 64:65],
                                          ones_h[:, 0:H].unsqueeze(2))
                vps = ps.tile([P, 512], F32, tag="mm", bufs=2, name="vps")
                for d in range(6):
                    nc.tensor.matmul(vps[:, 0:cw], xT[b][d][:, P * t:P * (t + 1)],
                                     wv_h[d][:, c0:c0 + cw],
                                     start=(d == 0), stop=(d == 5))
                nc.vector.tensor_copy(
                    v4[:, t % 2, 8 * ci:8 * ci + cw // HD, 0:HD],
                    vps[:, 0:cw].rearrange("p (h c) -> p h c", c=HD))

            def emit_proj_half(b, t, ci, queue=None):
                c0, cw = ((0, 512), (512, 256))[ci]
                pps = ps.tile([P, 512], F32, tag="mm", bufs=2, name="pps")
                for d in range(6):
                    nc.tensor.matmul(pps[:, 0:cw],
                                     at[b][d][:, P * t:P * (t + 1)],
                                     wp_h[d][:, c0:c0 + cw],
                                     start=(d == 0), stop=(d == 5))
                osb = sb.tile([P, 512], F16, tag="outs", bufs=2, name="osb")
                nc.vector.tensor_add(osb[:, 0:cw], pps[:, 0:cw],
                                     bfin_bc[:, c0:c0 + cw])
                (queue or nc.sync).dma_start(out_il[b, t][:, c0:c0 + cw],
                                             osb[:, 0:cw])

            # deferred normalization state: (emit recip+muls of pair p at
            # pair p+1's end, once the broadcast DMA round-trip has landed)
            pending_norm = [None]

            def flush_norm():
                if pending_norm[0] is not None:
                    pending_norm[0]()
                    pending_norm[0] = None

            def emit_attention(b, fillers, pops_sched):
                for nh in range(2):
                    n0 = 512 * nh
                    for jp in range(6):
                        pops = pops_sched[6 * nh + jp]
                        qt, kt = qk[b][jp], qk[b][6 + jp]
                        ot = [ps.tile([65, 512], F32, tag="ot", bufs=2,
                                      name="otps") for _ in range(2)]
                        es2 = []

                        def do_av(j):
                            # one DoubleRow matmul per head: K=256 (two
                            # 128-token planes per PE cell), fp8 operands
                            e2 = es2[j].rearrange("p (two x) -> p two x",
                                                  two=2)
                            v4 = vp[b][j].rearrange("p (two x) -> p two x",
                                                    two=2)
                            for hh in range(2):
                                h = 2 * jp + hh
                                nc.tensor.matmul(
                                    ot[hh][:],
                                    v4[:, :, 80 * h:80 * h + 65],
                                    e2[:, :, 512 * hh:512 * (hh + 1)],
                                    start=(j == 0), stop=(j == 3),
                                    perf_mode=DR)

                        for mb in range(4):
                            if mb > 0:
                                do_av(mb - 1)
                            sps2 = []
                            for mi in range(2):
                                m = 2 * mb + mi
                                sps = ps.tile([P, N], F32, tag="s", bufs=2,
                                              name="sps")
                                for hh in range(2):
                                    r0, r1 = HD * hh, HD * (hh + 1)
                                    nc.tensor.matmul(
                                        sps[:, 512 * hh:512 * (hh + 1)],
                                        kt[r0:r1, P * m:P * (m + 1)],
                                        qt[r0:r1, n0:n0 + 512],
                                        start=True, stop=True)
                                sps2.append(sps)
                            e2t = sb.tile([P, 2048], F8, tag="e", bufs=2,
                                          name="e2")
                            es2.append(e2t)
                            for mi in range(2):
                                nc.scalar.activation(
                                    e2t[:, 1024 * mi:1024 * (mi + 1)],
                                    sps2[mi][:], AF.Exp, scale=SCALE)
                            for _ in range(pops[mb]):
                                if fillers:
                                    fillers.pop(0)()
                        do_av(3)

                        last = b == 1 and nh == 1 and jp == 5
                        # previous pair's recip+muls first (its broadcast
                        # landed during this pair)
                        flush_norm()
                        if last:
                            # tail critical path: all on-chip (the DMA
                            # round-trip costs ~8us here and re-throttles
                            # HAM). O evac on ACT (idle now), denominator
                            # rows + recips on DVE, broadcast via two
                            # rank-1 PE matmuls into the dead sps banks.
                            # Warm MMs first: they run while the DVE chain
                            # computes, holding HAM at 8/8 for the tail.
                            keep_warm(10)
                            u2f = sb.tile([P, 512], F16, tag="u2f", bufs=1,
                                          name="u2f")
                            drf = sb.tile([1, 1024], F32, tag="drf", bufs=1,
                                          name="drf")
                            for hh in range(2):
                                nc.vector.tensor_copy(
                                    drf[0:1, 512 * hh:512 * (hh + 1)],
                                    ot[hh][64:65, :])
                                nc.scalar.copy(u2f[HD * hh:HD * (hh + 1), :],
                                               ot[hh][0:HD, :])
                            rrf = sb.tile([1, 1024], F32, tag="rrf", bufs=1,
                                          name="rrf")
                            nc.vector.reciprocal_approx_fast(out=rrf[:],
                                                             in_=drf[:])
                            rrh = sb.tile([1, 1024], F16, tag="rrh", bufs=1,
                                          name="rrh")
                            nc.vector.tensor_copy(rrh[:], rrf[:])
                            bcp = ps.tile([P, 512], F32, tag="s", bufs=2,
                                          name="bcp")
                            for hh in range(2):
                                nc.tensor.matmul(
                                    bcp[HD * hh:HD * (hh + 1), :],
                                    ones_h[0:1, 0:HD],
                                    rrh[0:1, 512 * hh:512 * (hh + 1)],
                                    start=True, stop=True)
                            nc.vector.tensor_mul(at[b][jp][:, n0:n0 + 512],
                                                 u2f[:], bcp[:])
                            continue
                        # evacuate O^T + denominator row per head (f32),
                        # then DMA the denominator rows out for broadcast
                        u2a = sb.tile([65, 512], F32, tag="u2a", bufs=2,
                                      name="u2a")
                        u2b = sb.tile([65, 512], F32, tag="u2b", bufs=2,
                                      name="u2b")
                        nc.vector.tensor_copy(u2a[:], ot[0][:])
                        nc.vector.tensor_copy(u2b[:], ot[1][:])
                        dr_d = dp.tile([1, 1024], F32, tag="dr_d", bufs=2,
                                       name="dr_d")
                        bca = sb.tile([HD, 512], F32, tag="bca", bufs=2,
                                      name="bca")
                        bcb = sb.tile([HD, 512], F32, tag="bcb", bufs=2,
                                      name="bcb")
                        # sync queue (HWDGE): the f32 broadcasts are 260KB
                        # per pair -- on gpsimd's SWDGE path that alone
                        # nearly saturates the queue and any added backlog
                        # stalls every pair's deferred norm
                        nc.sync.dma_start(dr_d[0:1, 0:512], u2a[64:65, :])
                        nc.sync.dma_start(dr_d[0:1, 512:1024],
                                          u2b[64:65, :])
                        nc.sync.dma_start(
                            bca[:], dr_d[0:1, 0:512].to_broadcast((HD, 512)))
                        nc.sync.dma_start(
                            bcb[:],
                            dr_d[0:1, 512:1024].to_broadcast((HD, 512)))

                        def norm(jp=jp, n0=n0, u2a=u2a, u2b=u2b,
                                 bca=bca, bcb=bcb):
                            rca = sb.tile([HD, 512], F32, tag="rca", bufs=2,
                                          name="rca")
                            rcb = sb.tile([HD, 512], F32, tag="rcb", bufs=2,
                                          name="rcb")
                            nc.vector.reciprocal_approx_fast(out=rca[:],
                                                             in_=bca[:])
                            nc.vector.reciprocal_approx_fast(out=rcb[:],
                                                             in_=bcb[:])
                            nc.vector.tensor_mul(
                                at[b][jp][0:HD, n0:n0 + 512],
                                u2a[0:HD, :], rca[:])
                            nc.vector.tensor_mul(
                                at[b][jp][HD:P, n0:n0 + 512],
                                u2b[0:HD, :], rcb[:])
                        pending_norm[0] = norm

            # ---------- prologue compute for attention-0 ----------
            # Only what gates pair 0: V t0/t1 (-> vp[0][0], needed by
            # AV(0) at mb1) + the pair-0 Q/K triple. Woven keep-warm MMs
            # hold HAM through the DMA-paced stretch. The remaining b0 V
            # groups front-load into fill0 (popped hard in pair 0 so
            # vp[0][j] lands just ahead of AV(j)).
            for t in range(2):
                for ci in range(2):
                    emit_v_group(0, t, ci)
                keep_warm(1)
            emit_qkv_group(0, 0, 0)
            emit_qkv_group(0, 6, 0)
            emit_qkv_group(0, 6, 1)

            # ---------- attention-0 fillers (56 pops) ----------
            fill0 = []
            fill0 += [lambda t=t, ci=ci: emit_v_group(0, t, ci)
                      for t in range(2, 8) for ci in range(2)]
            for jp in range(1, 6):
                fill0 += [lambda j=jp: emit_qkv_group(0, j, 0),
                          lambda j=jp + 6: emit_qkv_group(0, j, 0),
                          lambda j=jp + 6: emit_qkv_group(0, j, 1)]
            fill0 += [lambda j=jp: emit_qkv_group(0, j, 1) for jp in range(6)]
            fill0 += [lambda t=t, ci=ci: emit_v_group(1, t, ci)
                      for t in range(8) for ci in range(2)]
            for jp in range(2):
                fill0 += [lambda j=jp: emit_qkv_group(1, j, 0),
                          lambda j=jp + 6: emit_qkv_group(1, j, 0),
                          lambda j=jp + 6: emit_qkv_group(1, j, 1)]
            fill0 += [lambda: emit_qkv_group(1, 2, 0)]
            fill0 += [lambda: keep_warm(1)] * 7

            # pair 0 front-loads the 12 remaining b0-V units (vp[0][j]
            # lands just before AV(j) reads it) + the jp1 Q/K triple
            pops0 = [[2, 2, 4, 7]] + [[1, 1, 1, 1]] * 11
            emit_attention(0, fill0, pops0)
            for f in fill0:
                f()

            # ---------- attention-1 fillers (48 pops) ----------
            fill1 = [lambda: emit_qkv_group(1, 8, 0),
                     lambda: emit_qkv_group(1, 8, 1)]
            for jp in range(3, 6):
                fill1 += [lambda j=jp: emit_qkv_group(1, j, 0),
                          lambda j=jp + 6: emit_qkv_group(1, j, 0),
                          lambda j=jp + 6: emit_qkv_group(1, j, 1)]
            fill1 += [lambda j=jp: emit_qkv_group(1, j, 1) for jp in range(6)]
            fill1 += [lambda t=t, ci=ci: emit_proj_half(0, t, ci)
                      for t in range(8) for ci in range(2)]
            # b1 proj t0-3 needs at[1] nh0 incl. the deferred pair-5 mul
            # (flushed at pair 6) -> placed after pop 28
            fill1 += [lambda t=t, ci=ci: emit_proj_half(1, t, ci)
                      for t in range(4) for ci in range(2)]
            fill1 += [lambda: keep_warm(1)] * 7

            pops1 = [[1, 1, 1, 1]] * 12
            emit_attention(1, fill1, pops1)
            for f in fill1:
                f()

            # ---------- tail: b1 proj second half ----------
            # NOT gpsimd: its SWDGE path drains at ~1/3 the HWDGE rate
            tailq = [nc.sync, nc.scalar, nc.sync, nc.scalar,
                     nc.sync, nc.scalar, nc.sync, nc.scalar]
            qi = 0
            for t in range(4, 8):
                for ci in range(2):
                    emit_proj_half(1, t, ci, queue=tailq[qi])
                    qi += 1
    nc.compile()
    return nc


def _get_nc():
    if "nc" not in _CACHE:
        _CACHE["nc"] = _build()
    return _CACHE["nc"]


def _in_maps(x, W_qkv, b_qkv, W_proj, b_proj):
    x16 = np.ascontiguousarray(x, dtype=np.float16)
    # xT[b, j, p, 128*t + pp] = x[b, 8*pp + t, 128*j + p]
    xT = np.ascontiguousarray(
        x16.reshape(-1, P, 8, D).transpose(0, 3, 2, 1)
           .reshape(-1, 6, P, 8 * P))
    bq32 = np.asarray(b_qkv, dtype=np.float32)
    bqkvT = np.ascontiguousarray(bq32.reshape(18, P).T)
    bfin = (np.asarray(W_proj, dtype=np.float32).T @ bq32[2 * D:]
            + np.asarray(b_proj, dtype=np.float32)).reshape(1, D)
    bfin = np.ascontiguousarray(bfin, dtype=np.float32)
    wq16 = np.ascontiguousarray(W_qkv, dtype=np.float16)
    wp16 = np.ascontiguousarray(W_proj, dtype=np.float16)
    return [
        {
            "xT": xT[2 * i:2 * i + 2],
            "W_qkv": wq16,
            "bqkvT": bqkvT,
            "W_proj": wp16,
            "bfin": bfin,
        }
        for i in range(8)
    ]


def kernel(x, W_qkv, b_qkv, W_proj, b_proj):
    from concourse.bass_utils import run_bass_kernel_spmd

    nc = _get_nc()
    in_maps = _in_maps(x, W_qkv, b_qkv, W_proj, b_proj)
    res = run_bass_kernel_spmd(nc, in_maps, core_ids=list(range(8)))
    return np.concatenate(
        [r["out"].astype(np.float32) for r in res.results], axis=0)


# revision 23
# speedup vs baseline: 1.1855x; 1.0131x over previous
"""Multi-head self-attention (B=16, N=1024, D=768, H=12) on 8 TRN2 NeuronCores.

Data-parallel over batch (2 batches per core, weights replicated, no
collectives). v3: fp8 DoubleRow AV + host-side layout prep + batched
normalization. (v2: 354us, PE 92% busy; v1: 453.8us.)

Layout: token 8p+t lives at partition p, slot t (attention is
permutation-invariant over tokens; undone at the out DMA). xT arrives
PRE-TRANSPOSED from host ([6,128,1024] f16 per batch) -- the on-chip
transpose matmuls + their PSUM evacuations are gone. bqkvT ([128,18]
f32) and the fused proj bias (W_proj^T b_v + b_proj) also come from
host.

Attention per head pair: S^T = K Q^T (f16, two K=64 matmuls run
concurrently in the top/bottom PE halves), E = exp(S^T*scale) written
DIRECTLY AS FP8-E4M3 by ACT (values in (0, 12.6] -- far from the 240
clip; measured end-to-end rel err 1.6e-2 vs the 2e-2 gate, dominated
equally by the E and V casts -- Q/K/proj must stay f16: simulated
V-GEMM-in-f8 fails at 2.4e-2).  AV runs in DoubleRow perf mode: V_aug
(= [x W_v | ones-col], fp8, 65 cols per head) for m-steps 2j/2j+1 sits
in one [128, 2, 65] stationary tile (two k-planes per PE cell), E pair
in one [128, 2, 512] moving AP -> K=256 per matmul, HALF the AV slots
of f16. DoubleRow tile_size stays (128,128) = the filler GEMM mode, so
no extra array-reconfig tax (the measured dead end: 4 tile modes
cycling cost +50us at f16).

Normalization (the v2 chain burned ~3.7us DVE/pair in single-lane
[1,N] ops): the ones-row denominator is evacuated as row 64 of the
[65,512] f32 u2 PSUM-evac copy (free), DMA'd from SBUF through DRAM
into a [128,512] broadcast, then ONE 128-lane reciprocal + two muls.
recip/muls are deferred one pair so the DVE FIFO never parks on the
DMA round-trip. All f32 (reciprocal_approx_fast requires fp32 bit
layout).

Engine budget at 354us baseline: PE 331.8us busy (91.5%), ACT 218.4
(192 exps at (1024+352)/1.2 = 1147ns each -- the hard floor of this
decomposition: exp-batching to N=2048 needs 8 PSUM banks for sps
double-buffering, impossible next to ot+fillers), DVE 195.9. v3
targets: PE ~245 (DR-AV -36, transposes -5.5, bfinal -2.5), DVE ~150,
ACT ~212; both attention windows become ACT(exp)-bound at ~9.2us/pair.

Filler schedule: attention-0 hosts b0 Q/K groups + b1 V + early b1
Q/K triples (qk tags are per-batch now -- no cross-batch WAR);
attention-1 hosts remaining b1 Q/K + all b0 proj + b1 proj t0-3
(needs at[1] nh0 + its deferred mul -> placed after pop 28); tail is
b1 proj t4-7 with out-DMAs spread across sync/vector/scalar queues.
DMA issue costs ~0.6us of the ISSUING engine: weights/per-pair
denominator round-trips ride gpsimd; xT0 splits across sync/scalar/
vector for a parallel ~6us prologue; nothing issues from scalar after
the first exp (ACT is the floor).
"""

import numpy as np

_CACHE: dict = {}

P = 128
BL, N, D, H, HD = 2, 1024, 768, 12, 64
D3 = 3 * D
SCALE = float(HD) ** -0.5


def _build():
    import concourse.mybir as mybir
    import concourse.tile as tile
    from concourse import bacc

    dt = mybir.dt
    F32, F16, F8 = dt.float32, dt.float16, dt.float8e4
    AF = mybir.ActivationFunctionType
    DR = mybir.MatmulPerfMode.DoubleRow

    nc = bacc.Bacc("TRN2", target_bir_lowering=False, debug=False)
    xT_d = nc.dram_tensor("xT", [BL, 6, P, N], F16, kind="ExternalInput").ap()
    wqkv_d = nc.dram_tensor("W_qkv", [D, D3], F16, kind="ExternalInput").ap()
    bqkvT_d = nc.dram_tensor("bqkvT", [P, 18], F32, kind="ExternalInput").ap()
    wproj_d = nc.dram_tensor("W_proj", [D, D], F16, kind="ExternalInput").ap()
    bfin_d = nc.dram_tensor("bfin", [1, D], F32, kind="ExternalInput").ap()
    out_d = nc.dram_tensor("out", [BL, N, D], F16, kind="ExternalOutput").ap()
    # token-interleaved view: partition p, slot t <-> token 8p+t
    out_il = out_d.rearrange("b (p i) d -> b i p d", p=P)     # [2, 8, 128, 768]

    with tile.TileContext(nc) as tc:
        with tc.tile_pool(name="sb", bufs=1) as sb, \
             tc.tile_pool(name="dp", bufs=1, space="DRAM") as dp, \
             tc.tile_pool(name="ps", bufs=2, space="PSUM") as ps:

            # ---------- constants ----------
            ones_h = sb.tile([P, P], F16, tag="ones_h", bufs=1, name="ones_h")
            nc.vector.memset(ones_h[:], 1.0)
            warm_h = sb.tile([P, 512], F16, tag="warm", bufs=1, name="warm_h")
            nc.vector.memset(warm_h[:], 0.0)

            # ---------- input DMAs ----------
            bqkvT = sb.tile([P, 18], F32, tag="bqkvT", bufs=1, name="bqkvT")
            nc.sync.dma_start(bqkvT[:], bqkvT_d)
            xT = {b: [sb.tile([P, N], F16, tag=f"xT{b}_{j}", bufs=1,
                              name=f"xT{b}_{j}") for j in range(6)]
                  for b in range(BL)}
            wq_h, wv_h, wp_h = [], [], []
            # W_v first on gpsimd (V groups are the prologue compute)
            for d in range(6):
                t = sb.tile([P, D], F16, tag=f"wv{d}", bufs=1, name=f"wv{d}")
                nc.gpsimd.dma_start(t[:], wqkv_d[P * d:P * (d + 1), 2 * D:D3])
                wv_h.append(t)
            # xT0 split across sync/scalar queues (both idle in prologue;
            # DMA issue is restricted to gpsimd/sync/scalar). HWDGE issue
            # instructions BLOCK the issuing engine until the ring frees
            # (~one 384KB transfer = ~4us), so scalar carries only
            # prologue-critical transfers and is clear before the first exp.
            for d in range(6):
                (nc.sync if d < 3 else nc.scalar).dma_start(
                    xT[0][d][:], xT_d[0, d])
            # W_q/W_k: 3 on sync, 3 on scalar (all issued pre-exp)
            for d in range(6):
                t = sb.tile([P, 2 * D], F16, tag=f"wqkv{d}", bufs=1,
                            name=f"wqkv{d}")
                (nc.sync if d < 3 else nc.scalar).dma_start(
                    t[:], wqkv_d[P * d:P * (d + 1), 0:2 * D])
                wq_h.append(t)
            # xT1: 3 on sync, 3 on gpsimd (needed only mid-attention-0)
            for d in range(6):
                (nc.sync if d < 3 else nc.gpsimd).dma_start(
                    xT[1][d][:], xT_d[1, d])
            for d in range(6):
                t = sb.tile([P, D], F16, tag=f"wproj{d}", bufs=1,
                            name=f"wproj{d}")
                nc.gpsimd.dma_start(t[:], wproj_d[P * d:P * (d + 1), :])
                wp_h.append(t)
            bfin_bc = sb.tile([P, D], F32, tag="bfin_bc", bufs=1,
                              name="bfin_bc")
            nc.gpsimd.dma_start(bfin_bc[:], bfin_d.to_broadcast((P, D)))

            def keep_warm(n):
                # ~215ns each of dense matmul keeps/flips HAM to 8/8
                for wi in range(n):
                    wps = ps.tile([P, 512], F32, tag="mm", bufs=2, name="wps")
                    nc.tensor.matmul(wps[:], ones_h[:, 0:P], warm_h[:],
                                     start=True, stop=True)

            # PE warm-up bridges the DGE-ramp / DMA wait and flips HAM
            keep_warm(18)
            # pre-load the exp table set (~2.7us ACT) during the DMA wait
            escr = sb.tile([P, 16], F8, tag="escr", bufs=1, name="escr")
            nc.scalar.activation(escr[:], warm_h[:, 0:16], AF.Exp, scale=1.0)
            keep_warm(8)

            # ---------- tiles ----------
            # qk tags per-batch: b1 Q/K groups may run inside attention-0
            qk = {b: [sb.tile([P, N], F16, tag=f"qk{b}_{j}", bufs=1,
                              name=f"qk{b}_{j}") for j in range(12)]
                  for b in range(BL)}
            # V pairs, fp8, plane-major: [2 m-planes x 12 heads x 80] --
            # 80-wide head blocks (65 used) keep the DoubleRow weights AP
            # 16-byte aligned (plane stride 960, head offset 80h; walrus
            # ISA-checks the LDWEIGHTS access pattern)
            vp = {b: [sb.tile([P, 2 * H * 80], F8, tag=f"v{b}_{j}", bufs=1,
                              name=f"v{b}_{j}") for j in range(4)]
                  for b in range(BL)}
            at = {b: [sb.tile([P, N], F16, tag=f"at{b}_{j}", bufs=1,
                              name=f"at{j}") for j in range(6)]
                  for b in range(BL)}

            def emit_qkv_group(b, j, nh, tag="mm"):
                qps = ps.tile([P, 512], F32, tag=tag, bufs=2, name="qps")
                for d in range(6):
                    nc.tensor.matmul(qps[:], wq_h[d][:, P * j:P * (j + 1)],
                                     xT[b][d][:, 512 * nh:512 * (nh + 1)],
                                     start=(d == 0), stop=(d == 5))
                nc.vector.tensor_scalar_add(
                    qk[b][j][:, 512 * nh:512 * (nh + 1)], qps[:],
                    bqkvT[:, j:j + 1])

            def emit_v_group(b, t, ci):
                c0, cw = ((0, 512), (512, 256))[ci]
                v4 = vp[b][t // 2].rearrange("p (two h m) -> p two h m",
                                             two=2, m=80)
                if ci == 0:
                    # ones columns for this plane (the denominator rows)
                    nc.vector.tensor_copy(v4[:, t % 2, :, 64:65],
                                          ones_h[:, 0:H].unsqueeze(2))
                vps = ps.tile([P, 512], F32, tag="mm", bufs=2, name="vps")
                for d in range(6):
                    nc.tensor.matmul(vps[:, 0:cw], xT[b][d][:, P * t:P * (t + 1)],
                                     wv_h[d][:, c0:c0 + cw],
                                     start=(d == 0), stop=(d == 5))
                nc.vector.tensor_copy(
                    v4[:, t % 2, 8 * ci:8 * ci + cw // HD, 0:HD],
                    vps[:, 0:cw].rearrange("p (h c) -> p h c", c=HD))

            def emit_proj_half(b, t, ci, queue=None):
                c0, cw = ((0, 512), (512, 256))[ci]
                pps = ps.tile([P, 512], F32, tag="mm", bufs=2, name="pps")
                for d in range(6):
                    nc.tensor.matmul(pps[:, 0:cw],
                                     at[b][d][:, P * t:P * (t + 1)],
                                     wp_h[d][:, c0:c0 + cw],
                                     start=(d == 0), stop=(d == 5))
                osb = sb.tile([P, 512], F16, tag="outs", bufs=2, name="osb")
                nc.vector.tensor_add(osb[:, 0:cw], pps[:, 0:cw],
                                     bfin_bc[:, c0:c0 + cw])
                (queue or nc.sync).dma_start(out_il[b, t][:, c0:c0 + cw],
                                             osb[:, 0:cw])

            # deferred normalization state: (emit recip+muls of pair p at
            # pair p+1's end, once the broadcast DMA round-trip has landed)
            pending_norm = [None]

            def flush_norm():
                if pending_norm[0] is not None:
                    pending_norm[0]()
                    pending_norm[0] = None

            def emit_attention(b, fillers, pops_sched):
                for nh in range(2):
                    n0 = 512 * nh
                    for jp in range(6):
                        pops = pops_sched[6 * nh + jp]
                        qt, kt = qk[b][jp], qk[b][6 + jp]
                        ot = [ps.tile([65, 512], F32, tag="ot", bufs=2,
                                      name="otps") for _ in range(2)]
                        es2 = []

                        def do_av(j):
                            # one DoubleRow matmul per head: K=256 (two
                            # 128-token planes per PE cell), fp8 operands
                            e2 = es2[j].rearrange("p (two x) -> p two x",
                                                  two=2)
                            v4 = vp[b][j].rearrange("p (two x) -> p two x",
                                                    two=2)
                            for hh in range(2):
                                h = 2 * jp + hh
                                nc.tensor.matmul(
                                    ot[hh][:],
                                    v4[:, :, 80 * h:80 * h + 65],
                                    e2[:, :, 512 * hh:512 * (hh + 1)],
                                    start=(j == 0), stop=(j == 3),
                                    perf_mode=DR)

                        # AV deferred by TWO m-pairs: when emitted one pair
                        # behind, its exp dependency is still in flight and
                        # the tile scheduler splits the two QK slots around
                        # the AVs -- 4 64<->128 array reconfigs per mb
                        # (~430ns) instead of 2. At depth 2 the exp is long
                        # done, the emission order holds, and QK slots stay
                        # adjacent. Needs e2 bufs=3.
                        for mb in range(4):
                            sps2 = []
                            for mi in range(2):
                                m = 2 * mb + mi
                                sps = ps.tile([P, N], F32, tag="s", bufs=2,
                                              name="sps")
                                for hh in range(2):
                                    r0, r1 = HD * hh, HD * (hh + 1)
                                    nc.tensor.matmul(
                                        sps[:, 512 * hh:512 * (hh + 1)],
                                        kt[r0:r1, P * m:P * (m + 1)],
                                        qt[r0:r1, n0:n0 + 512],
                                        start=True, stop=True)
                                sps2.append(sps)
                            if mb > 1:
                                do_av(mb - 2)
                            e2t = sb.tile([P, 2048], F8, tag="e", bufs=3,
                                          name="e2")
                            es2.append(e2t)
                            for mi in range(2):
                                nc.scalar.activation(
                                    e2t[:, 1024 * mi:1024 * (mi + 1)],
                                    sps2[mi][:], AF.Exp, scale=SCALE)
                            for _ in range(pops[mb]):
                                if fillers:
                                    fillers.pop(0)()
                        do_av(2)
                        do_av(3)

                        last = b == 1 and nh == 1 and jp == 5
                        # previous pair's recip+muls first (its broadcast
                        # landed during this pair)
                        flush_norm()
                        if last:
                            # tail critical path: all on-chip (the DMA
                            # round-trip costs ~8us here and re-throttles
                            # HAM). O evac on ACT (idle now), denominator
                            # rows + recips on DVE, broadcast via two
                            # rank-1 PE matmuls into the dead sps banks.
                            # Warm MMs first: they run while the DVE chain
                            # computes, holding HAM at 8/8 for the tail.
                            keep_warm(10)
                            u2f = sb.tile([P, 512], F16, tag="u2f", bufs=1,
                                          name="u2f")
                            drf = sb.tile([1, 1024], F32, tag="drf", bufs=1,
                                          name="drf")
                            for hh in range(2):
                                nc.vector.tensor_copy(
                                    drf[0:1, 512 * hh:512 * (hh + 1)],
                                    ot[hh][64:65, :])
                                nc.scalar.copy(u2f[HD * hh:HD * (hh + 1), :],
                                               ot[hh][0:HD, :])
                            rrf = sb.tile([1, 1024], F32, tag="rrf", bufs=1,
                                          name="rrf")
                            nc.vector.reciprocal_approx_fast(out=rrf[:],
                                                             in_=drf[:])
                            rrh = sb.tile([1, 1024], F16, tag="rrh", bufs=1,
                                          name="rrh")
                            nc.vector.tensor_copy(rrh[:], rrf[:])
                            bcp = ps.tile([P, 512], F32, tag="s", bufs=2,
                                          name="bcp")
                            for hh in range(2):
                                nc.tensor.matmul(
                                    bcp[HD * hh:HD * (hh + 1), :],
                                    ones_h[0:1, 0:HD],
                                    rrh[0:1, 512 * hh:512 * (hh + 1)],
                                    start=True, stop=True)
                            nc.vector.tensor_mul(at[b][jp][:, n0:n0 + 512],
                                                 u2f[:], bcp[:])
                            continue
                        # evacuate O^T + denominator row per head (f32),
                        # then DMA the denominator rows out for broadcast
                        u2a = sb.tile([65, 512], F32, tag="u2a", bufs=2,
                                      name="u2a")
                        u2b = sb.tile([65, 512], F32, tag="u2b", bufs=2,
                                      name="u2b")
                        nc.vector.tensor_copy(u2a[:], ot[0][:])
                        nc.vector.tensor_copy(u2b[:], ot[1][:])
                        dr_d = dp.tile([1, 1024], F32, tag="dr_d", bufs=2,
                                       name="dr_d")
                        bca = sb.tile([HD, 512], F32, tag="bca", bufs=2,
                                      name="bca")
                        bcb = sb.tile([HD, 512], F32, tag="bcb", bufs=2,
                                      name="bcb")
                        # sync queue (HWDGE): the f32 broadcasts are 260KB
                        # per pair -- on gpsimd's SWDGE path that alone
                        # nearly saturates the queue and any added backlog
                        # stalls every pair's deferred norm
                        nc.sync.dma_start(dr_d[0:1, 0:512], u2a[64:65, :])
                        nc.sync.dma_start(dr_d[0:1, 512:1024],
                                          u2b[64:65, :])
                        nc.sync.dma_start(
                            bca[:], dr_d[0:1, 0:512].to_broadcast((HD, 512)))
                        nc.sync.dma_start(
                            bcb[:],
                            dr_d[0:1, 512:1024].to_broadcast((HD, 512)))

                        def norm(jp=jp, n0=n0, u2a=u2a, u2b=u2b,
                                 bca=bca, bcb=bcb):
                            rca = sb.tile([HD, 512], F32, tag="rca", bufs=2,
                                          name="rca")
                            rcb = sb.tile([HD, 512], F32, tag="rcb", bufs=2,
                                          name="rcb")
                            nc.vector.reciprocal_approx_fast(out=rca[:],
                                                             in_=bca[:])
                            nc.vector.reciprocal_approx_fast(out=rcb[:],
                                                             in_=bcb[:])
                            nc.vector.tensor_mul(
                                at[b][jp][0:HD, n0:n0 + 512],
                                u2a[0:HD, :], rca[:])
                            nc.vector.tensor_mul(
                                at[b][jp][HD:P, n0:n0 + 512],
                                u2b[0:HD, :], rcb[:])
                        pending_norm[0] = norm

            # ---------- prologue compute for attention-0 ----------
            # Only what gates pair 0: V t0/t1 (-> vp[0][0], needed by
            # AV(0) at mb1) + the pair-0 Q/K triple. Woven keep-warm MMs
            # hold HAM through the DMA-paced stretch. The remaining b0 V
            # groups front-load into fill0 (popped hard in pair 0 so
            # vp[0][j] lands just ahead of AV(j)).
            for t in range(2):
                for ci in range(2):
                    emit_v_group(0, t, ci)
                keep_warm(1)
            emit_qkv_group(0, 0, 0)
            emit_qkv_group(0, 6, 0)
            emit_qkv_group(0, 6, 1)

            # ---------- attention-0 fillers (56 pops) ----------
            fill0 = []
            fill0 += [lambda t=t, ci=ci: emit_v_group(0, t, ci)
                      for t in range(2, 8) for ci in range(2)]
            for jp in range(1, 6):
                fill0 += [lambda j=jp: emit_qkv_group(0, j, 0),
                          lambda j=jp + 6: emit_qkv_group(0, j, 0),
                          lambda j=jp + 6: emit_qkv_group(0, j, 1)]
            fill0 += [lambda j=jp: emit_qkv_group(0, j, 1) for jp in range(6)]
            fill0 += [lambda t=t, ci=ci: emit_v_group(1, t, ci)
                      for t in range(8) for ci in range(2)]
            for jp in range(2):
                fill0 += [lambda j=jp: emit_qkv_group(1, j, 0),
                          lambda j=jp + 6: emit_qkv_group(1, j, 0),
                          lambda j=jp + 6: emit_qkv_group(1, j, 1)]
            fill0 += [lambda: emit_qkv_group(1, 2, 0)]
            fill0 += [lambda: keep_warm(1)] * 7

            # pair 0 front-loads the 12 remaining b0-V units (vp[0][j]
            # lands just before AV(j) reads it) + the jp1 Q/K triple
            pops0 = [[2, 2, 4, 7]] + [[1, 1, 1, 1]] * 11
            emit_attention(0, fill0, pops0)
            for f in fill0:
                f()

            # ---------- attention-1 fillers (48 pops) ----------
            fill1 = [lambda: emit_qkv_group(1, 8, 0),
                     lambda: emit_qkv_group(1, 8, 1)]
            for jp in range(3, 6):
                fill1 += [lambda j=jp: emit_qkv_group(1, j, 0),
                          lambda j=jp + 6: emit_qkv_group(1, j, 0),
                          lambda j=jp + 6: emit_qkv_group(1, j, 1)]
            fill1 += [lambda j=jp: emit_qkv_group(1, j, 1) for jp in range(6)]
            fill1 += [lambda t=t, ci=ci: emit_proj_half(0, t, ci)
                      for t in range(8) for ci in range(2)]
            # b1 proj t0-3 needs at[1] nh0 incl. the deferred pair-5 mul
            # (flushed at pair 6) -> placed after pop 28
            fill1 += [lambda t=t, ci=ci: emit_proj_half(1, t, ci)
                      for t in range(4) for ci in range(2)]
            fill1 += [lambda: keep_warm(1)] * 7

            pops1 = [[1, 1, 1, 1]] * 12
            emit_attention(1, fill1, pops1)
            for f in fill1:
                f()

            # ---------- tail: b1 proj second half ----------
            # NOT gpsimd: its SWDGE path drains at ~1/3 the HWDGE rate
            tailq = [nc.sync, nc.scalar, nc.sync, nc.scalar,
                     nc.sync, nc.scalar, nc.sync, nc.scalar]
            qi = 0
            for t in range(4, 8):
                for ci in range(2):
                    emit_proj_half(1, t, ci, queue=tailq[qi])
                    qi += 1
    nc.compile()
    return nc


def _get_nc():
    if "nc" not in _CACHE:
        _CACHE["nc"] = _build()
    return _CACHE["nc"]


def _in_maps(x, W_qkv, b_qkv, W_proj, b_proj):
    x16 = np.ascontiguousarray(x, dtype=np.float16)
    # xT[b, j, p, 128*t + pp] = x[b, 8*pp + t, 128*j + p]
    xT = np.ascontiguousarray(
        x16.reshape(-1, P, 8, D).transpose(0, 3, 2, 1)
           .reshape(-1, 6, P, 8 * P))
    bq32 = np.asarray(b_qkv, dtype=np.float32)
    bqkvT = np.ascontiguousarray(bq32.reshape(18, P).T)
    bfin = (np.asarray(W_proj, dtype=np.float32).T @ bq32[2 * D:]
            + np.asarray(b_proj, dtype=np.float32)).reshape(1, D)
    bfin = np.ascontiguousarray(bfin, dtype=np.float32)
    wq16 = np.ascontiguousarray(W_qkv, dtype=np.float16)
    wp16 = np.ascontiguousarray(W_proj, dtype=np.float16)
    return [
        {
            "xT": xT[2 * i:2 * i + 2],
            "W_qkv": wq16,
            "bqkvT": bqkvT,
            "W_proj": wp16,
            "bfin": bfin,
        }
        for i in range(8)
    ]


def kernel(x, W_qkv, b_qkv, W_proj, b_proj):
    from concourse.bass_utils import run_bass_kernel_spmd

    nc = _get_nc()
    in_maps = _in_maps(x, W_qkv, b_qkv, W_proj, b_proj)
    res = run_bass_kernel_spmd(nc, in_maps, core_ids=list(range(8)))
    return np.concatenate(
        [r["out"].astype(np.float32) for r in res.results], axis=0)
